# revision 28
# baseline (speedup 1.0000x reference)
"""2-layer multi-head GAT on 8 TRN2 NeuronCores (Bass/Tile), v3.

Sharding: destination-node blocks. Core i owns nodes [i*NPC, (i+1)*NPC) and
all edges whose dst lands there, so edge softmax + aggregation are fully
core-local. z-tables are replicated via two AllGathers.

v3 vs v2: the per-edge s_dst dma_gather (half of all gather descriptors and
the top GpSimd consumer) is gone. Destination scores stay resident in SBUF
(s1_all/s2_all, written by the dense phases), and the per-edge expansion
s_dst[dst(e)] is a TensorE matmul qd = OT_c @ s_win per 128-edge chunk,
where OT_c is the transposed one-hot. OT is precomputed on the host (the
edge structure is static) and streamed from DRAM; the forward one-hot O is
still built on DVE via is_equal (bf16 now). Other fixes: leakyrelu moved to
ScalarE (Lrelu), PSUM->SBUF copies moved to ScalarE, all pathological
tensor_scalar ops replaced with tensor_tensor broadcasts, t2 table in bf16
with an embedded ones column so the L2 numerator row needs one multiply.
"""
import sys
sys.path.insert(0, "/opt/trn_rl_repo")

import numpy as np
import ml_dtypes

import concourse.bass as bass
import concourse.bacc as bacc
import concourse.tile as tile
import concourse.mybir as mybir
from concourse.bass_utils import run_bass_kernel_spmd
from concourse.masks import make_identity

F32 = mybir.dt.float32
BF16 = mybir.dt.bfloat16
I16 = mybir.dt.int16

NCORES = 8
HALF = 32768           # int16 gather index split
BFNP = ml_dtypes.bfloat16


def _round_up(x, m):
    return (x + m - 1) // m * m


# ----------------------------------------------------------------- host prep

def preprocess(h, src, dst, W1, a1, W2, a2):
    N, IN_DIM = h.shape
    HEADS, _, HID = W1.shape
    OUT = W2.shape[1]
    npc = N // NCORES
    rows = _round_up(npc + 1, 128)
    NW = rows // 128

    # weight folding (weights-only algebra)
    w1cat = np.transpose(W1, (1, 0, 2)).reshape(IN_DIM, HEADS * HID)
    w1s = np.stack([W1[hh] @ a1[hh, :HID] for hh in range(HEADS)], 1)
    w1d = np.stack([W1[hh] @ a1[hh, HID:] for hh in range(HEADS)], 1)
    wc1 = np.concatenate([w1cat, w1s, w1d], 1).astype(np.float32)
    wc2 = np.concatenate([W2, (W2 @ a2[:OUT])[:, None],
                          (W2 @ a2[OUT:])[:, None]], 1).astype(np.float32)
    # layer-2 input is elu(h1) = h1e - 1 with h1e = relu+exp(min(,0)); fold
    # the -1 through the matmul: z2 = h1e@wc2 - crow, crow = wc2.sum(0)
    crow = np.tile(wc2.sum(0)[None, :], (128, 1)).astype(np.float32)

    # piece-major global row numbering so each AllGather piece lands
    # contiguously in t*_full: gnew = 8*S[p] + core*sz[p] + (r - S[p])
    NT = rows // 128
    ptiles = [NT - 7 * (NT // 8)] + [NT // 8] * 7
    S = np.cumsum([0] + ptiles)[:-1] * 128
    SZ = np.array(ptiles) * 128
    pieces = [(int(S[p]), int(SZ[p])) for p in range(8)]

    core_of = dst // npc
    lr = src % npc
    pidx = np.searchsorted(S, lr, side="right") - 1
    gsrc_all = 8 * S[pidx] + (src // npc) * SZ[pidx] + (lr - S[pidx])

    pc = []
    for c in range(NCORES):
        m = core_of == c
        dstl = (dst[m] - c * npc).astype(np.int64)
        gsrc = gsrc_all[m].astype(np.int64)
        pc.append((dstl, gsrc))

    # per (core, window, half) counts -> static chunk structure
    low_cnt = np.zeros((NCORES, NW), dtype=np.int64)
    high_cnt = np.zeros((NCORES, NW), dtype=np.int64)
    for c in range(NCORES):
        dstl, gsrc = pc[c]
        w = dstl // 128
        hi = gsrc >= HALF
        np.add.at(low_cnt[c], w[~hi], 1)
        np.add.at(high_cnt[c], w[hi], 1)
    KL = np.maximum(1, np.ceil(low_cnt.max(0) / 128.0).astype(np.int64))
    KH = np.ceil(high_cnt.max(0) / 128.0).astype(np.int64)
    chunks = KL + KH
    bases = (np.concatenate([[0], np.cumsum(chunks)]) * 128).astype(np.int64)
    total_pos = int(bases[-1])

    eidx = np.zeros((NCORES, total_pos), dtype=np.int16)
    # dst-local window offset per position; -1 for padding slots (one-hot
    # column stays all-zero, so padding edges scatter nowhere)
    wloc_flat = np.full((NCORES, total_pos), -1.0, dtype=np.float32)
    for c in range(NCORES):
        dstl, gsrc = pc[c]
        w = dstl // 128
        hi = (gsrc >= HALF).astype(np.int64)
        key = w * 2 + hi
        order = np.argsort(key, kind="stable")
        ks = key[order]
        new = np.ones(len(ks), dtype=bool)
        new[1:] = ks[1:] != ks[:-1]
        starts = np.flatnonzero(new)
        lens = np.diff(np.append(starts, len(ks)))
        within = np.arange(len(ks)) - np.repeat(starts, lens)
        w_o, h_o = w[order], hi[order]
        pos = bases[w_o] + h_o * KL[w_o] * 128 + within
        eidx[c, pos] = (gsrc[order] - h_o * HALF).astype(np.int16)
        wloc_flat[c, pos] = (dstl[order] - w_o * 128).astype(np.float32)

    # wloc: [128, total/128] with partition = within-chunk slot
    wloc_t = np.ascontiguousarray(
        wloc_flat.reshape(NCORES, total_pos // 128, 128).transpose(0, 2, 1))
    # host-precomputed transposed one-hot: otab[j, pos] = (wloc[pos] == j)
    jj = np.arange(128, dtype=np.float32)
    otab = (wloc_flat[:, None, :] == jj[None, :, None]).astype(BFNP)

    windows = [(int(bases[w]), int(KL[w]), int(KH[w])) for w in range(NW)]
    struct = dict(
        N=N, E=src.shape[0], IN_DIM=IN_DIM, HEADS=HEADS, HID=HID, OUT=OUT,
        npc=npc, rows=rows, total_pos=total_pos, windows=windows,
        pieces=pieces,
    )

    def idx_tile(a):
        t = a.reshape(-1, 16).T.copy()
        return np.concatenate([t] * 8, 0)

    iota_pf = np.tile(np.arange(128, dtype=np.float32)[None, :],
                      (128, 1)).astype(BFNP)

    in_maps = []
    for c in range(NCORES):
        hs = np.zeros((rows, IN_DIM), dtype=np.float32)
        hs[:npc] = h[c * npc:(c + 1) * npc]
        in_maps.append({
            "h": hs,
            "eidx": idx_tile(eidx[c]),
            "wloc": wloc_t[c].astype(BFNP),
            "otab": otab[c],
            "iota": iota_pf,
            "wc1": wc1,
            "wc2": wc2,
            "crow": crow,
        })
    return struct, in_maps


# --------------------------------------------------------------- bass graph

def build(s):
    npc, rows, total_pos = s["npc"], s["rows"], s["total_pos"]
    windows, pieces = s["windows"], s["pieces"]
    IN_DIM, HEADS, HID, OUT = s["IN_DIM"], s["HEADS"], s["HID"], s["OUT"]
    ZC = HEADS * HID
    NW = rows // 128
    AF = mybir.ActivationFunctionType

    nc = bacc.Bacc("TRN2", target_bir_lowering=False, debug=False,
                   num_devices=NCORES, num_swdge_queues=4)

    h_in = nc.dram_tensor("h", [rows, IN_DIM], F32, kind="ExternalInput")
    eidx_in = nc.dram_tensor("eidx", [128, total_pos // 16], I16,
                             kind="ExternalInput")
    wloc_in = nc.dram_tensor("wloc", [128, total_pos // 128], BF16,
                             kind="ExternalInput")
    otab_in = nc.dram_tensor("otab", [128, total_pos], BF16,
                             kind="ExternalInput")
    iota_in = nc.dram_tensor("iota", [128, 128], BF16, kind="ExternalInput")
    wc1_in = nc.dram_tensor("wc1", [IN_DIM, ZC + 8], F32, kind="ExternalInput")
    wc2_in = nc.dram_tensor("wc2", [ZC, OUT + 2], F32, kind="ExternalInput")
    crow_in = nc.dram_tensor("crow", [128, OUT + 2], F32, kind="ExternalInput")
    out_ext = nc.dram_tensor("out", [rows, OUT], F32, kind="ExternalOutput")

    with tile.TileContext(nc) as tc:
        with (
            tc.tile_pool(name="dram", bufs=1, space="DRAM") as dram,
            tc.tile_pool(name="const", bufs=1) as const,
            tc.tile_pool(name="psum_tp", bufs=2, space="PSUM") as psum_tp,
        ):
            t1_loc = dram.tile([rows, 384], BF16)
            t1_full = nc.dram_tensor("t1_full_sh", [NCORES * rows, 384],
                                     BF16, kind="Internal",
                                     addr_space="Shared").ap()
            t2_loc = dram.tile([rows, 128], BF16)
            t2_full = nc.dram_tensor("t2_full_sh", [NCORES * rows, 128],
                                     BF16, kind="Internal",
                                     addr_space="Shared").ap()

            ident = const.tile([128, 128], F32)
            make_identity(nc, ident[:])
            wc1_t = const.tile([IN_DIM, ZC + 8], F32)
            nc.sync.dma_start(wc1_t[:], wc1_in[:])
            wc2a = const.tile([128, OUT + 2], F32)
            wc2b = const.tile([128, OUT + 2], F32)
            nc.sync.dma_start(wc2a[:], wc2_in[0:128, :])
            nc.sync.dma_start(wc2b[:], wc2_in[128:256, :])
            crow = const.tile([128, OUT + 2], F32)
            nc.sync.dma_start(crow[:], crow_in[:])
            iota = const.tile([128, 128], BF16)
            nc.sync.dma_start(iota[:], iota_in[:])
            eidx_t = const.tile([128, total_pos // 16], I16)
            nc.sync.dma_start(eidx_t[:], eidx_in[:])
            wloc_t = const.tile([128, total_pos // 128], BF16)
            nc.sync.dma_start(wloc_t[:], wloc_in[:])
            # score tables: dst scores per window, resident in SBUF
            s1_all = const.tile([128, NW, HEADS], BF16)
            s2_all = const.tile([128, NW], BF16)
            onesb = const.tile([128, 1], BF16)
            epsb = const.tile([128, HEADS], F32)
            nc.vector.memset(onesb[:], 1.0)
            nc.vector.memset(epsb[:], 1e-30)

            # ---------------- D1: z1 | s_src1 -> T1; s_dst1 -> s1_all -----
            with (
                tc.tile_pool(name="d1", bufs=4) as d1,
                tc.tile_pool(name="psum_d1", bufs=3, space="PSUM") as psum_d1,
            ):
                for t in range(NW):
                    ht = d1.tile([128, IN_DIM], F32, tag="ht")
                    nc.sync.dma_start(ht[:], h_in[t * 128:(t + 1) * 128, :])
                    hT_ps = psum_tp.tile([128, 128], F32, tag="tp")
                    nc.tensor.transpose(hT_ps[:], ht[:], ident[:])
                    hT = d1.tile([128, 128], F32, tag="hT")
                    nc.scalar.copy(hT[:], hT_ps[:])
                    zps = psum_d1.tile([128, ZC + 8], F32, tag="zp")
                    nc.tensor.matmul(zps[:], hT[:], wc1_t[:])

                    t1t = d1.tile([128, 384], BF16, tag="t1t")
                    nc.vector.tensor_copy(t1t[:, 0:ZC], zps[:, 0:ZC])
                    nc.vector.tensor_copy(
                        t1t[:, ZC:ZC + 8].bitcast(F32), zps[:, ZC:ZC + 4])
                    nc.vector.tensor_copy(
                        s1_all[:, t, :], zps[:, ZC + 4:ZC + 8])
                    nc.sync.dma_start(
                        t1_loc[t * 128:(t + 1) * 128, :], t1t[:])

            for (ps, sz) in pieces:
                nc.gpsimd.collective_compute(
                    "AllGather", mybir.AluOpType.bypass,
                    replica_groups=[list(range(NCORES))],
                    ins=[t1_loc[ps:ps + sz, :].opt()],
                    outs=[t1_full[8 * ps:8 * ps + 8 * sz, :].opt()],
                )

            # ------- L1 edge phase + fused D2, per 128-node window --------
            with (
                tc.tile_pool(name="l1", bufs=4) as l1,
                tc.tile_pool(name="l1m", bufs=3) as l1m,
                tc.tile_pool(name="d2", bufs=3) as d2,
                tc.tile_pool(name="psum_agg", bufs=3, space="PSUM") as psum_agg,
                tc.tile_pool(name="psum_qd", bufs=2, space="PSUM") as psum_qd,
                tc.tile_pool(name="psum_z2", bufs=1, space="PSUM") as psum_z2,
            ):
                qc = 0
                for wi, (base, KLw, KHw) in enumerate(windows):
                    C = KLw + KHw
                    bc = base // 128
                    g = l1.tile([128, C, 384], BF16, tag="g")
                    # split low/high each in two; spread over all 4 queues
                    parts = []
                    for c0, c1, tab in ((0, KLw, 0), (KLw, C, HALF)):
                        n = c1 - c0
                        if n >= 2:
                            mid = c0 + n // 2
                            parts += [(c0, mid, tab), (mid, c1, tab)]
                        elif n == 1:
                            parts.append((c0, c1, tab))
                    for c0, c1, tab in parts:
                        b0, b1 = base + c0 * 128, base + c1 * 128
                        nc.gpsimd.dma_gather(
                            g[:, c0:c1, :], t1_full[tab:, :],
                            eidx_t[:, b0 // 16:b1 // 16],
                            num_idxs=(c1 - c0) * 128,
                            num_idxs_reg=(c1 - c0) * 128,
                            elem_size=384, single_packet=False,
                            queue_num=qc % 4); qc += 1

                    ot = l1m.tile([128, C, 128], BF16, tag="ot")
                    nc.sync.dma_start(
                        ot[:], otab_in[:, base:base + C * 128])
                    o = l1m.tile([128, C, 128], BF16, tag="o")
                    nc.vector.tensor_tensor(
                        o[:],
                        wloc_t[:, bc:bc + C, None].to_broadcast((128, C, 128)),
                        iota[:, None, :].to_broadcast((128, C, 128)),
                        mybir.AluOpType.is_equal)

                    qd = psum_qd.tile([128, C, HEADS], F32, tag="qd")
                    for cc in range(C):
                        nc.tensor.matmul(
                            qd[:, cc, :], ot[:, cc, :], s1_all[:, wi, :],
                            start=True, stop=True)

                    q = l1.tile([128, C, HEADS], F32, tag="q")
                    nc.vector.tensor_add(
                        q[:], g[:, :, ZC:ZC + 8].bitcast(F32), qd[:])
                    # exp(leakyrelu(q)) == max(exp(q), exp(0.01*q)) -- keeps
                    # ScalarE on one activation table (no Lrelu<->Exp swaps)
                    ea = l1.tile([128, C, HEADS], BF16, tag="ea")
                    nc.scalar.activation(ea[:], q[:], AF.Exp)
                    eb = l1.tile([128, C, HEADS], BF16, tag="eb")
                    nc.scalar.activation(eb[:], q[:], AF.Exp, scale=0.01)
                    num = l1.tile([128, C, HEADS], BF16, tag="num")
                    nc.vector.tensor_max(num[:], ea[:], eb[:])

                    m = l1m.tile([128, C, ZC + HEADS], BF16, tag="m")
                    nc.vector.tensor_tensor(
                        m[:, :, 0:ZC].rearrange(
                            "p c (h x) -> p c h x", h=HEADS),
                        g[:, :, 0:ZC].rearrange(
                            "p c (h x) -> p c h x", h=HEADS),
                        num[:, :, :, None].to_broadcast((128, C, HEADS, HID)),
                        mybir.AluOpType.mult)
                    nc.vector.tensor_copy(m[:, :, ZC:ZC + HEADS], num[:])

                    agg = psum_agg.tile([128, ZC + HEADS], F32, tag="agg")
                    for cc in range(C):
                        nc.tensor.matmul(
                            agg[:], o[:, cc, :], m[:, cc, :],
                            start=(cc == 0), stop=(cc == C - 1))

                    # fused D2 for this window's 128 nodes
                    msum = d2.tile([128, ZC + HEADS], F32, tag="msum")
                    nc.scalar.copy(msum[:], agg[:])
                    nm = d2.tile([128, HEADS], F32, tag="nm")
                    nc.vector.tensor_add(
                        nm[:], msum[:, ZC:ZC + HEADS], epsb[:])
                    rec = d2.tile([128, HEADS], F32, tag="rec")
                    nc.vector.reciprocal(rec[:], nm[:])
                    h1 = d2.tile([128, ZC], F32, tag="h1")
                    nc.vector.tensor_tensor(
                        h1[:].rearrange("p (h x) -> p h x", h=HEADS),
                        msum[:, 0:ZC].rearrange("p (h x) -> p h x", h=HEADS),
                        rec[:, :, None].to_broadcast((128, HEADS, HID)),
                        mybir.AluOpType.mult)
                    # elu(x)+1 = relu(x) + exp(-relu(-x)); the -1 is folded
                    # into the z2 matmul via crow
                    relu = d2.tile([128, ZC], F32, tag="relu")
                    nc.scalar.activation(relu[:], h1[:], AF.Relu)
                    rn = d2.tile([128, ZC], F32, tag="rn")
                    nc.scalar.activation(rn[:], h1[:], AF.Relu, scale=-1.0)
                    ex = d2.tile([128, ZC], F32, tag="ex")
                    nc.scalar.activation(ex[:], rn[:], AF.Exp, scale=-1.0)
                    h1e = d2.tile([128, ZC], F32, tag="h1e")
                    nc.vector.tensor_add(h1e[:], relu[:], ex[:])

                    z2ps = psum_z2.tile([128, OUT + 2], F32, tag="z2p")
                    for kk in range(2):
                        tp = psum_tp.tile([128, 128], F32, tag="tp")
                        nc.tensor.transpose(
                            tp[:], h1e[:, kk * 128:(kk + 1) * 128], ident[:])
                        hT2 = d2.tile([128, 128], F32, tag="hT2")
                        nc.scalar.copy(hT2[:], tp[:])
                        nc.tensor.matmul(
                            z2ps[:], hT2[:], wc2a[:] if kk == 0 else wc2b[:],
                            start=(kk == 0), stop=(kk == 1))

                    r0, r1 = wi * 128, (wi + 1) * 128
                    t2t = d2.tile([128, 128], BF16, tag="t2t")
                    nc.vector.tensor_tensor(
                        t2t[:, 0:OUT], z2ps[:, 0:OUT], crow[:, 0:OUT],
                        mybir.AluOpType.subtract)
                    nc.vector.tensor_copy(t2t[:, OUT:OUT + 1], onesb[:])
                    nc.vector.tensor_tensor(
                        t2t[:, OUT + 1:OUT + 2], z2ps[:, OUT:OUT + 1],
                        crow[:, OUT:OUT + 1], mybir.AluOpType.subtract)
                    nc.vector.tensor_tensor(
                        s2_all[:, wi:wi + 1], z2ps[:, OUT + 1:OUT + 2],
                        crow[:, OUT + 1:OUT + 2], mybir.AluOpType.subtract)
                    nc.sync.dma_start(t2_loc[r0:r1, :], t2t[:])

            for (ps, sz) in pieces:
                nc.gpsimd.collective_compute(
                    "AllGather", mybir.AluOpType.bypass,
                    replica_groups=[list(range(NCORES))],
                    ins=[t2_loc[ps:ps + sz, :].opt()],
                    outs=[t2_full[8 * ps:8 * ps + 8 * sz, :].opt()],
                )

            # ------- L2 edge phase + output, per window -------------------
            with (
                tc.tile_pool(name="l2", bufs=4) as l2,
                tc.tile_pool(name="l2m", bufs=3) as l2m,
                tc.tile_pool(name="psum_a2", bufs=3, space="PSUM") as psum_a2,
                tc.tile_pool(name="psum_q2", bufs=2, space="PSUM") as psum_q2,
            ):
                qc = 0
                for wi, (base, KLw, KHw) in enumerate(windows):
                    C = KLw + KHw
                    bc = base // 128
                    g = l2.tile([128, C, 128], BF16, tag="g2")
                    parts = []
                    for c0, c1, tab in ((0, KLw, 0), (KLw, C, HALF)):
                        n = c1 - c0
                        if n >= 2:
                            mid = c0 + n // 2
                            parts += [(c0, mid, tab), (mid, c1, tab)]
                        elif n == 1:
                            parts.append((c0, c1, tab))
                    for c0, c1, tab in parts:
                        b0, b1 = base + c0 * 128, base + c1 * 128
                        nc.gpsimd.dma_gather(
                            g[:, c0:c1, :], t2_full[tab:, :],
                            eidx_t[:, b0 // 16:b1 // 16],
                            num_idxs=(c1 - c0) * 128,
                            num_idxs_reg=(c1 - c0) * 128,
                            elem_size=128, single_packet=False,
                            queue_num=qc % 4); qc += 1

                    ot = l2m.tile([128, C, 128], BF16, tag="ot2")
                    nc.sync.dma_start(
                        ot[:], otab_in[:, base:base + C * 128])
                    o = l2m.tile([128, C, 128], BF16, tag="o2")
                    nc.vector.tensor_tensor(
                        o[:],
                        wloc_t[:, bc:bc + C, None].to_broadcast((128, C, 128)),
                        iota[:, None, :].to_broadcast((128, C, 128)),
                        mybir.AluOpType.is_equal)

                    qd = psum_q2.tile([128, C, 1], F32, tag="qd2")
                    for cc in range(C):
                        nc.tensor.matmul(
                            qd[:, cc, :], ot[:, cc, :], s2_all[:, wi:wi + 1],
                            start=True, stop=True)

                    q = l2.tile([128, C, 1], F32, tag="q_2")
                    nc.vector.tensor_add(
                        q[:], g[:, :, OUT + 1:OUT + 2], qd[:])
                    ea = l2.tile([128, C, 1], BF16, tag="ea2")
                    nc.scalar.activation(ea[:], q[:], AF.Exp)
                    eb = l2.tile([128, C, 1], BF16, tag="eb2")
                    nc.scalar.activation(eb[:], q[:], AF.Exp, scale=0.01)
                    num = l2.tile([128, C, 1], BF16, tag="num2")
                    nc.vector.tensor_max(num[:], ea[:], eb[:])

                    m = l2m.tile([128, C, OUT + 1], BF16, tag="m2")
                    nc.vector.tensor_tensor(
                        m[:], g[:, :, 0:OUT + 1],
                        num[:].to_broadcast((128, C, OUT + 1)),
                        mybir.AluOpType.mult)

                    agg = psum_a2.tile([128, OUT + 1], F32, tag="agg2")
                    for cc in range(C):
                        nc.tensor.matmul(
                            agg[:], o[:, cc, :], m[:, cc, :],
                            start=(cc == 0), stop=(cc == C - 1))

                    msum = l2.tile([128, OUT + 1], F32, tag="bsum")
                    nc.scalar.copy(msum[:], agg[:])
                    nm = l2.tile([128, 1], F32, tag="bnm")
                    nc.vector.tensor_add(
                        nm[:], msum[:, OUT:OUT + 1], epsb[:, 0:1])
                    rec = l2.tile([128, 1], F32, tag="brec")
                    nc.vector.reciprocal(rec[:], nm[:])
                    otile = l2.tile([128, OUT], F32, tag="ot_out")
                    nc.vector.tensor_tensor(
                        otile[:].rearrange("p (a x) -> p a x", a=1),
                        msum[:, 0:OUT].rearrange("p (a x) -> p a x", a=1),
                        rec[:, :, None].to_broadcast((128, 1, OUT)),
                        mybir.AluOpType.mult)
                    nc.sync.dma_start(
                        out_ext[wi * 128:(wi + 1) * 128, :], otile[:])

    nc.compile()
    return nc


# ----------------------------------------------------------------- frontend

_CACHE = {}


def _run(h, src, dst, W1, a1, W2, a2, trace=False):
    struct, in_maps = preprocess(h, src, dst, W1, a1, W2, a2)
    key = (struct["N"], struct["E"], struct["total_pos"],
           tuple(struct["windows"]))
    if key not in _CACHE:
        _CACHE[key] = build(struct)
    nc = _CACHE[key]
    res = run_bass_kernel_spmd(nc, in_maps, core_ids=list(range(NCORES)),
                               trace=trace)
    npc = struct["npc"]
    out = np.concatenate(
        [res.results[c]["out"][:npc] for c in range(NCORES)], 0)
    return out.astype(np.float32), res


def kernel(h, src, dst, W1, a1, W2, a2):
    h = np.asarray(h, dtype=np.float32)
    src = np.asarray(src, dtype=np.int32)
    dst = np.asarray(dst, dtype=np.int32)
    W1 = np.asarray(W1, dtype=np.float32)
    a1 = np.asarray(a1, dtype=np.float32)
    W2 = np.asarray(W2, dtype=np.float32)
    a2 = np.asarray(a2, dtype=np.float32)
    out, _ = _run(h, src, dst, W1, a1, W2, a2, trace=False)
    return out
